# revision 50
# baseline (speedup 1.0000x reference)
"""Trainium2 Bass kernel for nn_CGIteration (CG tensor-product block combine).

Math (per sample n, per (l1,l2) input-block pair):
    out[n, M, p, q] = sum_{m1,m2} C[l1,l2,L,m1,m2,M] * x1_l1[n,m1,p] * x2_l2[n,m2,q]
with per-(L,S) output blocks concatenated along properties then flattened.

Kernel strategy (per core, 500 samples padded to 512), all-bf16 pipeline:
  1. Host prep: replicate x1/x2 into a [128-row, n*16] bf16 layout where row =
     (l1,l2,m1,m2); the 16 pairs split into two groups of exactly 128 rows
     each. Samples interleave pairwise in the columns (ILV=2) so every DVE
     operand has a packed 2-element innermost axis (2x DVE mode; tensor_tensor
     is capped at 2x because both read ports carry an operand).
  2. VectorE: z[row, (n,p,q)] = x1row[n,p] * x2row[n,q] via broadcast-AP
     tensor_mul (outer product over (p,q) per row per sample), bf16.
  3. TensorE: psum[(blockrow), cols] = CG_g[128, Mg].T @ z[128, 512] in bf16
     (full-rate; f32 PSUM accumulate). 512 matmuls total; each pays its own
     LDWEIGHTS (the build pins --enable-ldw-opt=false), which is why the
     M=128 "full-lane evac" regrouping (+256 matmuls) does not pay off.
  4. ScalarE/VectorE (~4:1 split): evacuate PSUM -> bf16 SBUF staging.
     Evac costs 1 col/cycle per copy regardless of partition count, so with
     z-gen this is ~400 us of engine work over the two engines - the floor.
  5. DMA: staging dumped to HBM per tile. The DGE spreads one dma_start over
     N SDMA engines, N = largest divisor of the partition count <= 16
     (68 -> 4 engines, 88 -> 11!), so each dump is split: rows [0:64]/[0:80]
     on the sync HWDGE (16 engines), the 4/8-row remainder via GpSimd SWDGE
     (keeps its per-dma ramp off the hot HWDGE rings). The last tile dumps
     only the real samples, in two chunks overlapping its final evacs.
     Host applies the final [n, 39936] column permutation during unshard.
"""

import numpy as np

import concourse.bass as bass
import concourse.mybir as mybir
from concourse.tile import TileContext
from concourse.bass_utils import run_bass_kernel_spmd

# ---------------------------------------------------------------- problem dims
L_MAX = 3
NL = 4            # input l = 0..3
Q = 16
N = 4000
NCORES = 8
NS = N // NCORES  # real samples per core (500)
NSP = 512         # padded samples per core
PQ = Q * Q        # 256

# ---------------------------------------------------------------- tunables
TILE_SAMPLES = 32          # samples per pipeline tile (divides NSP, mult of 8)
ILV = 2                    # sample interleave in z cols (packed innermost pair)
Z_DT = mybir.dt.bfloat16   # dtype of xz inputs / z / cg / staging
PS_COLS = 1024             # psum tile cols (2 banks); 2 matmuls of 512 each
Z_COLS = 4096              # cols per z subtile / tensor_mul instruction
Z_BUFS = 4                 # z slots per group
EVAC_DVE_MOD = (9, (0, 5))  # psum evacs with ctr%9 in {0,5} go to VectorE
ZGEN_GP_EVERY = 0          # every k-th z tensor_mul goes to GpSimd (0 = none)
HEAD_TILES = 2             # tiles covered by the fast head input load
PS_BUFS = 2                # psum slots per group (2g x 2slots x 2banks = 8)
# output staging row split: the big 16-divisible chunk engages all 16 SDMA
# engines (the DGE uses N = largest divisor of the partition count <= 16);
# the small remainder (4 / 8 rows) is issued via GpSimd's SWDGE so its
# per-dma ramp cost doesn't pile onto the same HWDGE rings.
OUT_SPLIT = {68: 64, 88: 80}

_NP_DT = {mybir.dt.float32: np.float32}
try:
    import ml_dtypes
    _NP_DT[mybir.dt.bfloat16] = ml_dtypes.bfloat16
except ImportError:
    pass


# ---------------------------------------------------------------- layout tables
def _combos():
    out = []
    for l1 in range(NL):
        for l2 in range(NL):
            for L in range(abs(l1 - l2), min(l1 + l2, L_MAX) + 1):
                out.append((l1, l2, L, (-1) ** (l1 + l2 + L)))
    return out


COMBOS = _combos()
KEYS = sorted({(L, S) for (_, _, L, S) in COMBOS})
BLOCKS = [[ci for ci, c in enumerate(COMBOS) if (c[2], c[3]) == k] for k in KEYS]
KEY_OFF = []
_off = 0
for k, blks in zip(KEYS, BLOCKS):
    KEY_OFF.append(_off)
    _off += (2 * k[0] + 1) * len(blks) * PQ
F = _off
assert F == 39936

# pair -> group coloring with K(group) = 128 both, M_out 68/88.
GROUP_PAIRS = [
    {(0, 0), (0, 1), (1, 0), (1, 3), (2, 0), (2, 2), (3, 1), (3, 3)},
    {(0, 2), (0, 3), (1, 1), (1, 2), (2, 1), (2, 3), (3, 0), (3, 2)},
]

# contraction rows per group: (l1, l2, m1, m2)
KROWS = []
for g in range(2):
    rows = []
    for (l1, l2) in sorted(GROUP_PAIRS[g]):
        for m1 in range(2 * l1 + 1):
            for m2 in range(2 * l2 + 1):
                rows.append((l1, l2, m1, m2))
    KROWS.append(rows)
assert len(KROWS[0]) == 128 and len(KROWS[1]) == 128


def _grp(ci):
    c = COMBOS[ci]
    return 0 if (c[0], c[1]) in GROUP_PAIRS[0] else 1


# output block-rows (psum partitions) per group, in global output order:
# within a group: (key, M-major, group-block-minor)
BROWS = [[], []]          # group -> list of (key_i, M, b_global, combo_i)
for key_i, ((L, S), blks) in enumerate(zip(KEYS, BLOCKS)):
    for M in range(2 * L + 1):
        for b, ci in enumerate(blks):
            g = _grp(ci)
            BROWS[g].append((key_i, M, b, ci))
MG = [len(BROWS[0]), len(BROWS[1])]
assert sum(MG) == 156


# ---------------------------------------------------------------- bass program
def _build_program():
    n2 = TILE_SAMPLES
    nt = NSP // n2
    f32 = mybir.dt.float32

    nc = bass.Bass()
    xz_dram = [
        [nc.dram_tensor(f"xz{x}_{g}", [128, NSP * Q], Z_DT, kind="ExternalInput")
         for x in (1, 2)]
        for g in range(2)
    ]
    cg_dram = [
        nc.dram_tensor(f"cg_{g}", [128, MG[g]], Z_DT, kind="ExternalInput")
        for g in range(2)
    ]
    out_dram = [
        nc.dram_tensor(f"out{g}", [nt, MG[g], n2 * PQ], Z_DT,
                       kind="ExternalOutput")
        for g in range(2)
    ]

    nh = n2 // ILV                   # sample-hyperplanes per tile
    zh = Z_COLS // (PQ * ILV)        # hyperplanes per z subtile
    n_zsub = n2 * PQ // Z_COLS
    n_ps_per_z = Z_COLS // PS_COLS

    with TileContext(nc) as tc:
        with tc.tile_pool(name="consts", bufs=1) as cpool, \
             tc.tile_pool(name="xin", bufs=1) as xpool, \
             tc.tile_pool(name="zp", bufs=Z_BUFS) as zpool, \
             tc.tile_pool(name="stg", bufs=2) as spool, \
             tc.tile_pool(name="ps", bufs=PS_BUFS, space="PSUM") as ppool:

            # whole input resident in SBUF (4 x 16 KB/partition bf16), loaded
            # in two stages so tile 0 can start after a small head transfer;
            # the head loads go first so z-gen starts as early as possible
            hc = HEAD_TILES * n2 * Q
            xhead = [[None, None], [None, None]]
            xtail = [[None, None], [None, None]]
            for ht in range(HEAD_TILES):      # tile 0's loads issue first
                for g in range(2):
                    for xi in range(2):
                        eng = nc.sync if g == 0 else nc.scalar
                        xh = xpool.tile([128, n2 * Q], Z_DT,
                                        tag=f"xh{xi}{g}{ht}",
                                        name=f"xh{xi}{g}{ht}")
                        c0 = ht * n2 * Q
                        eng.dma_start(out=xh[:],
                                      in_=xz_dram[g][xi][:, c0:c0 + n2 * Q])
                        if xhead[g][xi] is None:
                            xhead[g][xi] = []
                        xhead[g][xi].append(xh)
            cg_t = []
            for g in range(2):
                ct = cpool.tile([128, MG[g]], Z_DT, tag=f"cg{g}", name=f"cg{g}")
                nc.sync.dma_start(out=ct[:], in_=cg_dram[g][:])
                cg_t.append(ct)
            for g in range(2):
                for xi in range(2):
                    eng = nc.sync if g == 0 else nc.scalar
                    xt_ = xpool.tile([128, NSP * Q - hc], Z_DT,
                                     tag=f"xt{xi}{g}", name=f"xt{xi}{g}")
                    eng.dma_start(out=xt_[:], in_=xz_dram[g][xi][:, hc:])
                    xtail[g][xi] = xt_

            evac_ctr = 0
            zgen_ctr = 0
            for t in range(nt):
                if t < HEAD_TILES:
                    xt = [[xhead[g][xi][t][:]
                           for xi in range(2)] for g in range(2)]
                else:
                    toff = (t - HEAD_TILES) * n2 * Q
                    xt = [[xtail[g][xi][:, toff:toff + n2 * Q]
                           for xi in range(2)] for g in range(2)]

                n_real = min(n2, NS - t * n2)          # real samples in tile
                real_cols = n_real * PQ
                stg = []
                for g in range(2):
                    x1v = xt[g][0].rearrange("p (h a i) -> p h a i",
                                             h=nh, a=Q, i=ILV)
                    x1v = x1v[:, :, :, None, :].broadcast_to([128, nh, Q, Q, ILV])
                    x2v = xt[g][1].rearrange("p (h q i) -> p h q i",
                                             h=nh, q=Q, i=ILV)
                    x2v = x2v[:, :, None, :, :].broadcast_to([128, nh, Q, Q, ILV])

                    st_full = spool.tile([128, n2 * PQ], Z_DT,
                                         tag=f"st{g}", name=f"st{g}_{t}")
                    st = st_full[:MG[g]]
                    stg.append(st)
                    st_v = st.rearrange("m (p2 c) -> m p2 c", c=PS_COLS)

                    for zi in range(n_zsub):
                        if zi * Z_COLS >= real_cols:
                            continue
                        z = zpool.tile([128, Z_COLS], Z_DT,
                                       tag=f"z{g}", name=f"z{g}_{t}_{zi}")
                        zv = z[:].rearrange("p (h a q i) -> p h a q i",
                                            h=zh, a=Q, q=Q, i=ILV)
                        h0 = zi * zh
                        zgen_ctr += 1
                        if ZGEN_GP_EVERY and zgen_ctr % ZGEN_GP_EVERY == 0:
                            zeng = nc.gpsimd
                        else:
                            zeng = nc.vector
                        zeng.tensor_mul(
                            out=zv,
                            in0=x1v[:, h0:h0 + zh],
                            in1=x2v[:, h0:h0 + zh],
                        )
                        for pi in range(n_ps_per_z):
                            ps_g = zi * n_ps_per_z + pi   # psum idx within tile
                            # skip pad-only psum blocks of the last tile
                            # (samples 500-511; host discards them anyway)
                            if ps_g * PS_COLS >= real_cols:
                                continue
                            pt = ppool.tile([MG[g], PS_COLS], f32,
                                            tag=f"ps{g}",
                                            name=f"ps{g}_{t}_{zi}_{pi}")
                            for mmi in range(PS_COLS // 512):
                                c0 = pi * PS_COLS + mmi * 512
                                nc.tensor.matmul(
                                    out=pt[:, mmi * 512:(mmi + 1) * 512],
                                    lhsT=cg_t[g][:], rhs=z[:, c0:c0 + 512],
                                    start=True, stop=True,
                                )
                            evac_ctr += 1
                            mod, hits = EVAC_DVE_MOD
                            if evac_ctr % mod in hits:
                                nc.vector.tensor_copy(
                                    out=st_v[:, ps_g], in_=pt[:])
                            else:
                                nc.scalar.copy(out=st_v[:, ps_g], in_=pt[:])

                            # last two tiles: dump in chunks right behind
                            # the evacs so the final DMA drain overlaps them
                            if t >= nt - 2:
                                bnd = (real_cols // (2 * PS_COLS)) * PS_COLS
                                seg = None
                                if (ps_g + 1) * PS_COLS == bnd:
                                    seg = (0, bnd)
                                elif (ps_g + 1) * PS_COLS >= real_cols:
                                    seg = (bnd, real_cols)
                                if seg:
                                    cut = OUT_SPLIT[MG[g]]
                                    c0, c1 = seg
                                    nc.sync.dma_start(
                                        out=out_dram[g][t][:cut, c0:c1],
                                        in_=stg[g][:cut, c0:c1])
                                    nc.gpsimd.dma_start(
                                        out=out_dram[g][t][cut:, c0:c1],
                                        in_=stg[g][cut:, c0:c1])

                # dump staging (real cols only): big 16-divisible chunk on
                # sync HWDGE, small remainder on gpsimd SWDGE
                if t >= nt - 2:
                    continue
                for g in range(2):
                    cut = OUT_SPLIT[MG[g]]
                    nc.sync.dma_start(out=out_dram[g][t][:cut, :real_cols],
                                      in_=stg[g][:cut, :real_cols])
                    nc.gpsimd.dma_start(out=out_dram[g][t][cut:, :real_cols],
                                        in_=stg[g][cut:, :real_cols])
    return nc


def _split_excess_waits(nc, max_waits=1):
    """The walrus build in this image accepts at most one sync wait per
    instruction; Tile's tail drain carries one wait per active proc. Hoist
    excess waits onto same-engine NOPs inserted just before the offender
    (sequential on the engine, so semantics are unchanged)."""
    ctr = 0
    for b in nc.m.functions[0].blocks:
        insts = b.instructions
        new = []
        changed = False
        for inst in insts:
            si = inst.sync_info
            waits = list(si.on_wait) if (si and si.on_wait) else []
            if len(waits) > max_waits:
                head, waits = waits[:-max_waits], waits[-max_waits:]
                for w in head:
                    ctr += 1
                    nop = mybir.InstNoOp(
                        name=f"waitsplit-{ctr}", engine=inst.engine,
                        ins=[], outs=[],
                        sync_info=mybir.SyncInfo(on_wait=[w], on_update=[]),
                    )
                    new.append(nop)
                inst.sync_info = mybir.SyncInfo(
                    on_wait=waits, on_update=list(si.on_update))
                changed = True
            new.append(inst)
        if changed:
            insts[:] = new
    return ctr


_PROGRAM = None


def _get_program():
    global _PROGRAM
    if _PROGRAM is None:
        _PROGRAM = _build_program()
        _split_excess_waits(_PROGRAM)
    return _PROGRAM


# ---------------------------------------------------------------- host prep
def _prep_inputs(x1, x2, cg):
    """Build per-core in_maps. x1/x2: lists of [N, 2l+1, Q] f32. cg: table."""
    np_dt = _NP_DT[Z_DT]
    in_maps = [dict() for _ in range(NCORES)]

    for g in range(2):
        for xi, xsrc in ((1, x1), (2, x2)):
            arr = np.zeros((128, NCORES * NSP, Q), dtype=np.float32)
            view = arr.reshape(128, NCORES, NSP, Q)
            for r, (l1, l2, m1, m2) in enumerate(KROWS[g]):
                src = xsrc[l1][:, m1, :] if xi == 1 else xsrc[l2][:, m2, :]
                view[r, :, :NS, :] = src.reshape(NCORES, NS, Q)
            # pack cols as (n_hi, j, n_lo) with n = n_hi*ILV + n_lo
            packed = np.ascontiguousarray(
                arr.reshape(128, NCORES * NSP // ILV, ILV, Q)
                   .transpose(0, 1, 3, 2)
            ).reshape(128, NCORES * NSP * Q).astype(np_dt)
            for c in range(NCORES):
                in_maps[c][f"xz{xi}_{g}"] = np.ascontiguousarray(
                    packed[:, c * NSP * Q:(c + 1) * NSP * Q])

    for g in range(2):
        cgm = np.zeros((128, MG[g]), dtype=np.float32)
        row_of = {}
        for r, (r1, r2, m1, m2) in enumerate(KROWS[g]):
            row_of[(r1, r2, m1, m2)] = r
        for j, (key_i, M, b, ci) in enumerate(BROWS[g]):
            l1, l2, L, S = COMBOS[ci]
            for m1 in range(2 * l1 + 1):
                for m2 in range(2 * l2 + 1):
                    cgm[row_of[(l1, l2, m1, m2)], j] = cg[l1, l2, L, m1, m2, M]
        cgm = cgm.astype(np_dt)
        for c in range(NCORES):
            in_maps[c][f"cg_{g}"] = cgm

    return in_maps


_PERMS = None


def _out_perms():
    """perms[g][j*PQ + c] = output column of staging element (row j, col c)."""
    global _PERMS
    if _PERMS is None:
        perms = []
        for g in range(2):
            p = np.empty(MG[g] * PQ, dtype=np.int64)
            for j, (key_i, M, b, ci) in enumerate(BROWS[g]):
                nb = len(BLOCKS[key_i])
                base = KEY_OFF[key_i] + (M * nb + b) * PQ
                p[j * PQ:(j + 1) * PQ] = np.arange(base, base + PQ)
            perms.append(p)
        _PERMS = perms
    return _PERMS


# ---------------------------------------------------------------- entry points
def run(inputs, trace=False, trace_kwargs=None):
    x1 = [np.asarray(inputs[f"x1_l{l}"], dtype=np.float32) for l in range(NL)]
    x2 = [np.asarray(inputs[f"x2_l{l}"], dtype=np.float32) for l in range(NL)]
    cg = np.asarray(inputs["cg_coeffs"], dtype=np.float32)
    in_maps = _prep_inputs(x1, x2, cg)
    nc = _get_program()
    res = run_bass_kernel_spmd(
        nc, in_maps, list(range(NCORES)),
        trace=trace, **(trace_kwargs or {}),
    )
    out = np.empty((N, F), dtype=np.float32)
    perms = _out_perms()
    n2 = TILE_SAMPLES
    nt = NSP // n2
    nh = n2 // ILV
    for c in range(NCORES):
        rows = slice(c * NS, (c + 1) * NS)
        for g in range(2):
            # [NT, Mg, nh*PQ*ILV] -> [NT, nh, ILV, Mg, PQ] -> [NSP, Mg*PQ]
            a = np.asarray(res.results[c][f"out{g}"], dtype=np.float32)
            a = a.reshape(nt, MG[g], nh, PQ, ILV)
            a = np.ascontiguousarray(a.transpose(0, 2, 4, 1, 3))
            a = a.reshape(NSP, MG[g] * PQ)[:NS]
            out[rows, perms[g]] = a
    return out, res


def kernel(**inputs):
    out, _ = run(inputs)
    return out
